# revision 39
# baseline (speedup 1.0000x reference)
"""Multi-head attention (B=4, S=2048, D=1024, H=16) on 8 Trainium2 NeuronCores.

Sharding: batch (4-way data parallel) x head-group (2-way tensor parallel).
Core c handles batch c//2, heads [8*(c%2), 8*(c%2)+8).  Each core computes a
partial output [S, D] (its heads' contribution through its Wo row-slice); the
host sums the two partials per batch.

v2 schedule, built around the fact that ScalarE (exp over S*S*H/2 = 33.5M
elements per core, 256 ACTIVATEs at ~1.13us) is the pacing engine:
  - fine-grained prioritized input DMA so the first logits matmul runs ~7us
    in (pair-0 W slices + seq-block-0 of x^T first), with the exp table
    preloaded via a dummy activation during the DMA wait.
  - per key-chunk: logits pair (row-tiled K=64, both heads concurrent) ->
    exp -> E@V pair for chunk g-2 interleaved right behind, so the PE never
    batches EV after the logits stream (that starved ACT at combo ends).
  - softmax denominators: two serial bf16 accumulator chains (keys 0:1024,
    1024:2048) updated one DVE add per chunk, then a 4-matmul ones-reduction
    into PSUM at the combo tail, reciprocal straight off PSUM in one DVE op.
  - projections/Wo are split into <=4-matmul fill units, paced ~1 unit per
    chunk during the ACT-paced phase (budget 2/chunk while building the
    initial QKV inventory).
"""

import os

os.environ.setdefault("MYCRO_LOCAL_CACHE", "1")

from contextlib import ExitStack

import numpy as np
import ml_dtypes

import concourse.bacc as bacc
import concourse.mybir as mybir
import concourse.tile as tile

BF = mybir.dt.bfloat16
F32 = mybir.dt.float32
BF_NP = ml_dtypes.bfloat16

B, S, D, H = 4, 2048, 1024, 16
DEPTH = D // H          # 64
HPC = 8                 # heads per core
FPC = HPC * DEPTH       # 512 features per core
P = 128
CH = D // P             # 8 contraction chunks for the projections
NK = S // P             # 16 key chunks
NQ = S // 512           # 4 q chunks

_NC_CACHE = {}


def _emit(ctx: ExitStack, tc, xt_d, wq_d, wk_d, wv_d, wo_d, eb_d, out_d):
    nc = tc.nc
    Exp = mybir.ActivationFunctionType.Exp

    const = ctx.enter_context(tc.tile_pool(name="const", bufs=1))
    wpool = ctx.enter_context(tc.tile_pool(name="wpool", bufs=1))
    xpool = ctx.enter_context(tc.tile_pool(name="xpool", bufs=1))
    qkpool = ctx.enter_context(tc.tile_pool(name="qkpool", bufs=1))
    vpool = ctx.enter_context(tc.tile_pool(name="vpool", bufs=1))
    epool = ctx.enter_context(tc.tile_pool(name="epool", bufs=12))
    accpool = ctx.enter_context(tc.tile_pool(name="accpool", bufs=2))
    atpool = ctx.enter_context(tc.tile_pool(name="atpool", bufs=2))
    stpool = ctx.enter_context(tc.tile_pool(name="stpool", bufs=3))
    stApool = ctx.enter_context(tc.tile_pool(name="stApool", bufs=8))
    rpool = ctx.enter_context(tc.tile_pool(name="rpool", bufs=2))
    # PSUM budget (8 banks): scores 2x[128,1024] = 4, attn 2x[128,512] = 2,
    # misc (qkv/wo/dp/rb) 2x[128,512] = 2.
    ps_sc = ctx.enter_context(tc.tile_pool(name="ps_sc", bufs=2, space="PSUM"))
    ps_at = ctx.enter_context(tc.tile_pool(name="ps_at", bufs=2, space="PSUM"))
    ps_ms = ctx.enter_context(tc.tile_pool(name="ps_ms", bufs=2, space="PSUM"))

    ones = const.tile([1, DEPTH], F32)
    nc.vector.memset(ones, 1.0)
    ones128 = const.tile([P, 1], BF)
    nc.vector.memset(ones128, 1.0)
    ones64 = const.tile([P, 64], BF)
    nc.vector.memset(ones64, 1.0)
    # preload the exp table set on ScalarE while input DMAs run
    dummy = const.tile([1, 1], F32)
    nc.scalar.activation(dummy, ones[0:1, 0:1], Exp)

    bb_sb = const.tile([P, NK], F32)   # raw bias, chunked [key%128, chunk]

    # ---- input tiles, one per prioritized DMA piece ----
    # wq/wk: host layout [P, pair, CH, 128] -> per-pair tiles (2KB contig)
    wq_sb = [wpool.tile([P, CH, P], BF, name=f"wq{p}") for p in range(4)]
    wk_sb = [wpool.tile([P, CH, P], BF, name=f"wk{p}") for p in range(4)]
    # xt: host layout [P, 4, CH, 512] (s-block major) -> per (s, c-half) tiles
    xt_sb = [[xpool.tile([P, CH // 2, 512], BF, name=f"xt{s}_{h}")
              for h in range(2)] for s in range(NQ)]
    wv_sb = [wpool.tile([P, CH // 2, FPC], BF, name=f"wv{h}") for h in range(2)]
    wo_sb = [wpool.tile([P, 2, D], BF, name=f"wo{h}") for h in range(2)]

    # DMA priority order, round-robin over sync+gpsimd (keep the ScalarE
    # queue free: a DMA trigger there would delay the first exps).
    dq = (nc.sync, nc.gpsimd)
    dmas = [(wq_sb[0], wq_d[:, 0]),                    # sync
            (xt_sb[0][0], xt_d[:, 0, 0:4]),            # gpsimd
            (wk_sb[0], wk_d[:, 0]),                    # sync
            (xt_sb[0][1], xt_d[:, 0, 4:8]),            # gpsimd
            (bb_sb, eb_d)]
    for s in range(1, NQ):
        dmas += [(xt_sb[s][h], xt_d[:, s, 4 * h:4 * h + 4]) for h in range(2)]
    dmas += [(wv_sb[h], wv_d[:, 4 * h:4 * h + 4]) for h in range(2)]
    for p in range(1, 4):
        dmas += [(wq_sb[p], wq_d[:, p]), (wk_sb[p], wk_d[:, p])]
    dmas += [(wo_sb[h], wo_d[:, 2 * h:2 * h + 2]) for h in range(2)]
    for i, (dst, src) in enumerate(dmas):
        dq[i % 2].dma_start(out=dst, in_=src)

    # ---- phase 0: projections as 2x4-matmul fill units ----
    QT = qkpool.tile([P, HPC // 2, S], BF)   # [2 heads x 64 depth, pair, seq]
    KT = qkpool.tile([P, HPC // 2, S], BF)
    V = vpool.tile([P, NK, FPC], BF)         # [key%128, chunk, head*64+depth]

    # dependency marks: consumers may only be EMITTED after producers are
    v_done = [False] * NK
    kt_done = [[False] * NQ for _ in range(4)]
    qt_done = [[False] * NQ for _ in range(4)]

    # Fill units are popped in ATOMIC thunk pairs (both halves back-to-back)
    # so the 2-deep "ms" PSUM ring can never wrap onto a half-accumulated
    # tile: every allocation's full use (matmuls + drain copy) is emitted
    # before the next-next allocation.
    def qkt_units(w_sb, dst, pair, sc, marks):
        ps_box = []

        def half(h, w_sb=w_sb, dst=dst, pair=pair, sc=sc):
            if h == 0:
                ps_box.append(ps_ms.tile([P, 512], F32, tag="ms", name="ms_ps"))
            ps = ps_box[0]
            for cc in range(4 * h, 4 * h + 4):
                nc.tensor.matmul(
                    ps,
                    lhsT=w_sb[pair][:, cc, :],
                    rhs=xt_sb[sc][h][:, cc % 4, :],
                    start=(cc == 0),
                    stop=(cc == CH - 1),
                )
            if h == 1:
                nc.vector.tensor_copy(
                    dst[:, pair, 512 * sc:512 * (sc + 1)], ps
                )
                marks[pair][sc] = True
        return [(4, lambda: half(0)), (4, lambda: half(1))]

    def v_units(sb):
        ps_box = []

        def half(h, sb=sb):
            if h == 0:
                ps_box.append(ps_ms.tile([P, 512], F32, tag="ms", name="ms_ps"))
            ps = ps_box[0]
            for cc in range(4 * h, 4 * h + 4):
                nc.tensor.matmul(
                    ps,
                    lhsT=xt_sb[sb // 4][h][:, cc % 4,
                                           128 * (sb % 4):128 * (sb % 4) + 128],
                    rhs=wv_sb[h][:, cc % 4, :],
                    start=(cc == 0),
                    stop=(cc == CH - 1),
                )
            if h == 1:
                nc.vector.tensor_copy(V[:, sb, :], ps)
                v_done[sb] = True
        return [(4, lambda: half(0)), (4, lambda: half(1))]

    # PE warm-up during the input-DMA wait: ~300 tiny matmuls keep the HAM
    # activity monitor at K=8/8 so the first real matmuls run at 2.4 GHz.
    warm = ps_ms.tile([P, 64], F32, tag="ms", name="warm")
    for _ in range(160):
        nc.tensor.matmul(warm[0:1, :], lhsT=ones128, rhs=ones64,
                         start=True, stop=True)
    # seed: what the very first logits block needs, emitted eagerly
    for _, u in qkt_units(wq_sb, QT, 0, 0, qt_done):
        u()
    for _, u in qkt_units(wk_sb, KT, 0, 0, kt_done):
        u()

    # fill queue, priority-ordered (V early: EV(combo0) consumes it live)
    fill_q = []
    fill_q += v_units(0) + v_units(1)
    fill_q += qkt_units(wk_sb, KT, 0, 1, kt_done)
    fill_q += v_units(2) + v_units(3)
    fill_q += qkt_units(wk_sb, KT, 0, 2, kt_done)
    fill_q += v_units(4) + v_units(5)
    fill_q += qkt_units(wk_sb, KT, 0, 3, kt_done)
    fill_q += v_units(6) + v_units(7)
    fill_q += qkt_units(wq_sb, QT, 1, 0, qt_done)
    fill_q += qkt_units(wk_sb, KT, 1, 0, kt_done)
    for sb in range(8, 12):
        fill_q += v_units(sb)
    fill_q += qkt_units(wk_sb, KT, 1, 1, kt_done)
    fill_q += qkt_units(wk_sb, KT, 1, 2, kt_done)
    for sb in range(12, 16):
        fill_q += v_units(sb)
    fill_q += qkt_units(wk_sb, KT, 1, 3, kt_done)
    for pp in range(2, 4):
        fill_q += qkt_units(wq_sb, QT, pp, 0, qt_done)
        for sc in range(NQ):
            fill_q += qkt_units(wk_sb, KT, pp, sc, kt_done)
    deferred_qt = {
        qc - 1: [u for pp in range(HPC // 2)
                 for u in qkt_units(wq_sb, QT, pp, qc, qt_done)]
        for qc in range(1, NQ)
    }

    fill_pos = [0]
    fill_mms = [0]
    total_mms = [0]

    def pop_thunks(n):
        while n > 0 and fill_pos[0] < len(fill_q):
            for _ in range(2):   # both halves, atomically
                m, u = fill_q[fill_pos[0]]
                u()
                fill_pos[0] += 1
                fill_mms[0] += m
            n -= 1

    def pop_mms(n):
        while n > 0 and fill_pos[0] < len(fill_q):
            n -= fill_q[fill_pos[0]][0] + fill_q[fill_pos[0] + 1][0]
            pop_thunks(1)

    # ---- phases 1+2: attention + output projection ----
    # Cross-combo software pipeline with EV at lag 8: each chunk emits
    # [logits pair; exp; one tail item of the previous combo; EV(g-8);
    # quota-paced fills].  The tail of combo k (EV 8..15 and one atomic
    # denominator/normalize item) drains during chunks 0..8 of combo k+1.
    EVLAG = 8

    def wo_units(qb, n, attnT, q0):
        ps_box = []

        def half(h, qb=qb, n=n, attnT=attnT, q0=q0):
            if h == 0:
                ps_box.append(ps_ms.tile([P, 512], F32, tag="ms", name="ms_ps"))
            po = ps_box[0]
            for pair in (2 * h, 2 * h + 1):
                nc.tensor.matmul(
                    po,
                    lhsT=attnT[:, pair, 128 * qb:128 * (qb + 1)],
                    rhs=wo_sb[h][:, pair - 2 * h, 512 * n:512 * (n + 1)],
                    start=(pair == 0),
                    stop=(pair == HPC // 2 - 1),
                )
            if h == 1:
                st = stpool.tile([P, 512], F32, tag="st")
                nc.vector.tensor_copy(st, po)
                qq = q0 + 128 * qb
                nc.sync.dma_start(
                    out=out_d[qq:qq + 128, 512 * n:512 * (n + 1)], in_=st
                )
        return [(2, lambda: half(0)), (2, lambda: half(1))]

    stA_tiles = {}
    combos = [(qc, pair) for qc in range(NQ) for pair in range(HPC // 2)]
    attnTs = {}
    wo_mark = {}
    tail = {"items": [], "pos": 0}

    def drain_tail(n=1):
        while n > 0 and tail["pos"] < len(tail["items"]):
            tail["items"][tail["pos"]]()
            tail["pos"] += 1
            n -= 1

    for k, (qc, pair) in enumerate(combos):
        q0 = 512 * qc
        if pair == 0:
            fill_q.extend(deferred_qt.pop(qc, []))
            # attnT ring reuse (bufs=2): before allocating attnT(qc), the
            # consumers of attnT(qc-2) (its Wo units) must all be emitted.
            if qc - 2 in wo_mark:
                while fill_pos[0] < wo_mark[qc - 2]:
                    pop_thunks(1)
            attnTs[qc] = atpool.tile([P, HPC // 2, 512], BF, tag="attnT",
                                     name="attnT")
        total_mms[0] = sum(m for m, _ in fill_q[fill_pos[0]:])
        while not qt_done[pair][qc] and fill_pos[0] < len(fill_q):
            pop_thunks(1)
        attnT = attnTs[qc]
        hA, hB = 2 * pair, 2 * pair + 1
        atP = ps_at.tile([P, 512], F32, tag="at")
        e_ts = []
        acc = [None, None]   # two bf16 accumulator chains (key halves)

        def ev(g, atP=atP, hA=hA, hB=hB, e_ts=e_ts):
            # E@V col-packed: head A -> psum 0:64, head B -> 64:128
            nc.tensor.matmul(
                atP[0:DEPTH, :],
                lhsT=V[:, g, DEPTH * hA:DEPTH * (hA + 1)],
                rhs=e_ts[g][:, 0:512],
                start=(g == 0), stop=(g == NK - 1),
                tile_position=(0, 0), skip_group_check=True,
            )
            nc.tensor.matmul(
                atP[DEPTH:P, :],
                lhsT=V[:, g, DEPTH * hB:DEPTH * (hB + 1)],
                rhs=e_ts[g][:, 512:1024],
                start=(g == 0), stop=(g == NK - 1),
                tile_position=(0, DEPTH), skip_group_check=True,
            )

        def chain_add(g, acc=acc, e_ts=e_ts):
            c = g // 8
            if g % 8 == 0:
                return  # chain seeds on g%8==1 below
            if g % 8 == 1:
                acc[c] = accpool.tile([P, 1024], BF, tag=f"acc{c}",
                                      name=f"acc{c}")
                nc.vector.tensor_add(acc[c], e_ts[g - 1], e_ts[g])
            else:
                nc.vector.tensor_add(acc[c], acc[c], e_ts[g])

        ev_next = [0]

        def drain_ev(upto, ev=ev, chain_add=chain_add, ev_next=ev_next):
            while ev_next[0] <= upto:
                g = ev_next[0]
                while not v_done[g] and fill_pos[0] < len(fill_q):
                    pop_thunks(1)
                ev(g)
                chain_add(g)
                ev_next[0] += 1

        for g in range(NK):
            k0 = 128 * g
            if g % 4 == 0:
                while not kt_done[pair][g // 4] and fill_pos[0] < len(fill_q):
                    pop_thunks(1)
            sc_t = ps_sc.tile([P, 1024], F32, tag="sc")
            # two K=64 heads row-packed (base partitions 0 / 64)
            nc.tensor.matmul(
                sc_t[:, 0:512],
                lhsT=KT[0:DEPTH, pair, k0:k0 + 128],
                rhs=QT[0:DEPTH, pair, q0:q0 + 512],
                start=True, stop=True,
            )
            nc.tensor.matmul(
                sc_t[:, 512:1024],
                lhsT=KT[DEPTH:P, pair, k0:k0 + 128],
                rhs=QT[DEPTH:P, pair, q0:q0 + 512],
                start=True, stop=True,
            )
            e_t = epool.tile([P, 1024], BF, tag="e")
            # bias folded into exp: exp(l + bias[key]); bias is per
            # partition (= key) so one [P,1] AP serves both head halves
            nc.scalar.activation(e_t, sc_t, Exp, bias=bb_sb[:, g:g + 1])
            e_ts.append(e_t)
            drain_tail(1)
            drain_ev(g - EVLAG)
            # self-smoothing fill quota: spread remaining fill matmuls
            # evenly over remaining chunks
            rem_chunks = (len(combos) - k) * NK - g
            quota = (total_mms[0] - fill_mms[0]) / max(1, rem_chunks)
            if k < 12:
                # under-fill PE-paced early combos; late ACT-paced combos
                # absorb the leftovers in what would otherwise be PE idle
                quota *= 0.8
            if quota > 0:
                pop_mms(max(1, int(quota + 0.5)))
        drain_tail(len(tail["items"]))   # finish any previous-combo leftovers

        def mk_tail(k=k, qc=qc, pair=pair, q0=q0, drain_ev=drain_ev,
                    atP=atP, attnT=attnT, acc=acc):
            accf_box = []

            def t_ev(g):
                if g < NK - 1:
                    return lambda: drain_ev(g)

                def last():
                    drain_ev(g)
                    accf = accpool.tile([P, 1024], BF, tag="accf")
                    nc.vector.tensor_add(accf, acc[0], acc[1])
                    accf_box.append(accf)
                return last

            def t_den():
                # denominators: ones-matmuls over both accumulator chains
                # into one shared PSUM tile (rows 0/32); reciprocal straight
                # off PSUM row 0 (row 32 must stage through SBUF: approx-
                # recip mis-reads PSUM at base partition 32); broadcast
                # matmuls then overwrite the same tile; normalize-mul.
                # Emitted as ONE atomic item so no fill allocation can wrap
                # the 2-deep "ms" ring onto the live tile.
                accf = accf_box[0]
                T = ps_ms.tile([P, 512], F32, tag="ms", name="ms_dp")
                nc.tensor.matmul(T[0:1, :], lhsT=ones128,
                                 rhs=accf[:, 0:512],
                                 start=True, stop=True,
                                 tile_position=(0, 0),
                                 skip_group_check=True)
                nc.tensor.matmul(T[32:33, :], lhsT=ones128,
                                 rhs=accf[:, 512:1024],
                                 start=True, stop=True,
                                 tile_position=(0, 32),
                                 skip_group_check=True)
                rA = rpool.tile([1, 512], F32, tag="rA")
                rB = rpool.tile([1, 512], F32, tag="rB")
                dB = rpool.tile([1, 512], F32, tag="dB")
                nc.vector.reciprocal_approx_fast(rA, T[0:1, :])
                nc.vector.tensor_copy(dB, T[32:33, :])
                nc.vector.reciprocal_approx_fast(rB, dB)
                pop_thunks(1)   # one thunk covers the reciprocal latency
                nc.tensor.matmul(T[0:DEPTH, :], lhsT=ones, rhs=rA,
                                 start=True, stop=True, tile_position=(0, 0),
                                 skip_group_check=True)
                nc.tensor.matmul(T[DEPTH:P, :], lhsT=ones, rhs=rB,
                                 start=True, stop=True,
                                 tile_position=(0, DEPTH),
                                 skip_group_check=True)
                # tensor_tensor reads at most one PSUM operand: stage in SBUF
                rs = rpool.tile([P, 512], F32, tag="rb_sb")
                nc.vector.tensor_copy(rs, T)
                nc.vector.tensor_mul(attnT[:, pair, :], atP, rs)
                if pair == HPC // 2 - 1 and qc < NQ - 1:
                    for qb in range(4):
                        for n in range(2):
                            fill_q.extend(wo_units(qb, n, attnT, q0))
                    wo_mark[qc] = len(fill_q)
                if pair == 1 and qc == NQ - 1:
                    # last qc: Wo pairs 0-1 go out early as fills, partials
                    # parked in SBUF; pairs 2-3 merge on-chip at the end.
                    def wo_a(qb, n, attnT=attnT):
                        def u():
                            po = ps_ms.tile([P, 512], F32, tag="ms",
                                            name="ms_ps")
                            for pr in (0, 1):
                                nc.tensor.matmul(
                                    po,
                                    lhsT=attnT[:, pr, 128 * qb:128 * qb + 128],
                                    rhs=wo_sb[0][:, pr, 512 * n:512 * n + 512],
                                    start=(pr == 0), stop=(pr == 1),
                                )
                            stA = stApool.tile([P, 512], F32, tag="stA",
                                               name="stA")
                            nc.vector.tensor_copy(stA, po)
                            stA_tiles[(qb, n)] = stA
                        return (2, u)

                    for qb in range(4):
                        for n in range(2):
                            fill_q.append(wo_a(qb, n))

            return [t_ev(g) for g in range(NK - EVLAG, NK)] + [t_den]

        if k < len(combos) - 1:
            tail["items"] = mk_tail()
            tail["pos"] = 0
        else:
            for t in mk_tail():
                t()
            pop_thunks(len(fill_q))   # ensure all partA units are emitted
            for qb in range(4):
                for n in range(2):
                    po = ps_ms.tile([P, 512], F32, tag="ms", name="ms_ps")
                    for pr in (2, 3):
                        nc.tensor.matmul(
                            po,
                            lhsT=attnT[:, pr, 128 * qb:128 * qb + 128],
                            rhs=wo_sb[1][:, pr - 2, 512 * n:512 * n + 512],
                            start=(pr == 2), stop=(pr == 3),
                        )
                    st = stpool.tile([P, 512], F32, tag="st")
                    nc.vector.tensor_add(st, stA_tiles[(qb, n)], po)
                    qq = q0 + 128 * qb
                    nc.sync.dma_start(
                        out=out_d[qq:qq + 128, 512 * n:512 * (n + 1)], in_=st
                    )
    pop_thunks(len(fill_q))
    if os.environ.get("KDBG"):
        nc.sync.dma_start(out=_DBG["qt"], in_=QT)
        nc.sync.dma_start(out=_DBG["kt"], in_=KT)
        nc.sync.dma_start(out=_DBG["v"], in_=V)


_DBG = {}


def _build():
    nc = bacc.Bacc("TRN2", target_bir_lowering=False, debug=False)
    xt = nc.dram_tensor("xt", [P, NQ, CH, 512], BF, kind="ExternalInput").ap()
    wq = nc.dram_tensor("wq", [P, 4, CH, P], BF, kind="ExternalInput").ap()
    wk = nc.dram_tensor("wk", [P, 4, CH, P], BF, kind="ExternalInput").ap()
    wv = nc.dram_tensor("wv", [P, CH, FPC], BF, kind="ExternalInput").ap()
    wo = nc.dram_tensor("wo", [P, HPC // 2, D], BF, kind="ExternalInput").ap()
    eb = nc.dram_tensor("eb", [P, NK], F32, kind="ExternalInput").ap()
    out = nc.dram_tensor("out", [S, D], F32, kind="ExternalOutput").ap()
    if os.environ.get("KDBG"):
        _DBG["qt"] = nc.dram_tensor("dbg_qt", [P, HPC // 2, S], BF,
                                    kind="ExternalOutput").ap()
        _DBG["kt"] = nc.dram_tensor("dbg_kt", [P, HPC // 2, S], BF,
                                    kind="ExternalOutput").ap()
        _DBG["v"] = nc.dram_tensor("dbg_v", [P, NK, FPC], BF,
                                   kind="ExternalOutput").ap()

    with tile.TileContext(nc) as tc:
        with ExitStack() as ctx:
            _emit(ctx, tc, xt, wq, wk, wv, wo, eb, out)
    nc.compile()
    return nc


def get_nc():
    if "nc" not in _NC_CACHE:
        _NC_CACHE["nc"] = _build()
    return _NC_CACHE["nc"]


def _in_maps(x, bias, Wq, Wk, Wv, Wo):
    x = np.asarray(x, dtype=np.float32)
    bias = np.asarray(bias, dtype=np.float32)
    maps = []

    def pmajor(a, chunks):
        # [chunks*128, F] -> partition-major [128, chunks, F]
        return np.ascontiguousarray(
            a.reshape(chunks, P, a.shape[-1]).swapaxes(0, 1)
        )

    for core in range(8):
        b, grp = core // 2, core % 2
        cols = slice(FPC * grp, FPC * (grp + 1))
        # xt: [128, s-block, c-chunk, 512]
        xt_pm = pmajor(np.asarray(x[b]).T.astype(BF_NP), CH)  # [128, CH, S]
        xt = np.ascontiguousarray(
            xt_pm.reshape(P, CH, NQ, 512).transpose(0, 2, 1, 3)
        )
        # wq/wk: [128, pair, CH, 128]
        wq_pm = pmajor(
            (np.asarray(Wq)[:, cols] * (DEPTH ** -0.5)).astype(BF_NP), CH
        )  # [128, CH, FPC]
        wq = np.ascontiguousarray(
            wq_pm.reshape(P, CH, 4, P).transpose(0, 2, 1, 3)
        )
        wk_pm = pmajor(np.asarray(Wk)[:, cols].astype(BF_NP), CH)
        wk = np.ascontiguousarray(
            wk_pm.reshape(P, CH, 4, P).transpose(0, 2, 1, 3)
        )
        wv = pmajor(np.asarray(Wv)[:, cols].astype(BF_NP), CH)
        wo = pmajor(np.asarray(Wo)[cols, :].astype(BF_NP), HPC // 2)
        eb = np.ascontiguousarray(
            bias[b, 0, 0].astype(np.float32).reshape(NK, P).T
        )  # raw bias, [128 = key%128, 16 = key chunk]
        maps.append(
            {"xt": xt, "wq": wq, "wk": wk, "wv": wv, "wo": wo, "eb": eb}
        )
    return maps


def _get_exec():
    """Cached jitted SPMD executable mirroring bass2jax.run_bass_via_pjrt,
    without donation (our kernel writes every output element) so repeated
    calls can reuse persistent device buffers for timing."""
    if "exec" in _NC_CACHE:
        return _NC_CACHE["exec"]
    import jax
    import concourse.mybir as _mybir
    from concourse.bass2jax import (
        _bass_exec_p,
        install_neuronx_cc_hook,
        partition_id_tensor,
    )
    from jax.experimental.shard_map import shard_map
    from jax.sharding import Mesh, NamedSharding, PartitionSpec

    install_neuronx_cc_hook()
    nc = get_nc()
    n_cores = 8
    part_name = nc.partition_id_tensor.name if nc.partition_id_tensor else None
    in_names, out_names, out_avals = [], [], []
    for alloc in nc.m.functions[0].allocations:
        if not isinstance(alloc, _mybir.MemoryLocationSet):
            continue
        name = alloc.memorylocations[0].name
        if alloc.kind == "ExternalInput":
            if name != part_name:
                in_names.append(name)
        elif alloc.kind == "ExternalOutput":
            out_names.append(name)
            out_avals.append(
                jax.core.ShapedArray(
                    tuple(alloc.tensor_shape), _mybir.dt.np(alloc.dtype)
                )
            )
    n_params = len(in_names)
    all_names = in_names + out_names
    if part_name is not None:
        all_names = all_names + [part_name]

    def _body(*args):
        operands = list(args)
        if part_name is not None:
            operands.append(partition_id_tensor())
        return tuple(
            _bass_exec_p.bind(
                *operands,
                out_avals=tuple(out_avals),
                in_names=tuple(all_names),
                out_names=tuple(out_names),
                lowering_input_output_aliases=(),
                sim_require_finite=True,
                sim_require_nnan=True,
                nc=nc,
            )
        )

    devices = jax.devices()[:n_cores]
    mesh = Mesh(np.asarray(devices), ("core",))
    nshard = NamedSharding(mesh, PartitionSpec("core"))
    sharded = jax.jit(
        shard_map(
            _body,
            mesh=mesh,
            in_specs=(PartitionSpec("core"),) * (n_params + len(out_names)),
            out_specs=(PartitionSpec("core"),) * len(out_names),
            check_rep=False,
        ),
        keep_unused=True,
    )
    zeros = [
        jax.device_put(
            np.zeros((n_cores * a.shape[0], *a.shape[1:]), a.dtype), nshard
        )
        for a in out_avals
    ]
    _NC_CACHE["exec"] = (sharded, in_names, out_names, out_avals, nshard, zeros)
    return _NC_CACHE["exec"]


def _execute(maps):
    import jax

    sharded, in_names, out_names, out_avals, nshard, zeros = _get_exec()
    concat_in = [
        jax.device_put(
            np.concatenate([np.asarray(m[name]) for m in maps], axis=0), nshard
        )
        for name in in_names
    ]
    outs = sharded(*concat_in, *zeros)
    return concat_in, outs, out_names, out_avals


def run(x, bias, Wq, Wk, Wv, Wo, trace=False):
    """Returns (full_output [B,S,D] f32, per-core outs)."""
    maps = _in_maps(x, bias, Wq, Wk, Wv, Wo)
    _, outs, out_names, out_avals = _execute(maps)
    per_core = np.asarray(outs[out_names.index("out")]).reshape(8, S, D)
    full = np.empty((B, S, D), dtype=np.float32)
    for b in range(B):
        full[b] = per_core[2 * b] + per_core[2 * b + 1]
    return full, per_core


def bench(x, bias, Wq, Wk, Wv, Wo, iters=20):
    """Amortized per-execution wall time (ns) over pipelined dispatches."""
    import jax
    import time

    maps = _in_maps(x, bias, Wq, Wk, Wv, Wo)
    sharded, in_names, out_names, out_avals, nshard, zeros = _get_exec()
    concat_in = [
        jax.device_put(
            np.concatenate([np.asarray(m[name]) for m in maps], axis=0), nshard
        )
        for name in in_names
    ]
    outs = sharded(*concat_in, *zeros)  # warmup / compile
    jax.block_until_ready(outs)
    t0 = time.perf_counter()
    for _ in range(iters):
        outs = sharded(*concat_in, *zeros)
    jax.block_until_ready(outs)
    dt = (time.perf_counter() - t0) / iters
    return int(dt * 1e9)


def kernel(x, bias, Wq, Wk, Wv, Wo):
    return run(x, bias, Wq, Wk, Wv, Wo)[0]
